# revision 4
# baseline (speedup 1.0000x reference)
"""NT-Xent loss (B=4096, D=128, T=0.07) on 8 Trainium2 NeuronCores.

Strategy (one SPMD Bass program, 8 cores):
  - Host: z = concat(z_i, z_j) [8192,128], scale by 1/sqrt(T), transpose to
    zT [128, 8192].  Core c receives zT rotated left by c*1024 columns so its
    own 1024 rows sit at columns 0..1023 -> the self-sim diagonal block and
    the positive-pair diagonal block land at compile-time-constant column
    offsets on every core (one uniform SPMD program, no partition-id logic).
  - Device, per 128-row tile t (8 tiles per core), two passes over the same
    [128, 8192] similarity slab (PE matmul is cheap; PSUM egress is not):
      pass 1: PE 17 matmuls -> PSUM quarters [128,2048]; the self-diag block
              gets -1e5*I added via an extra accumulating matmul (identity x
              identity trick) so no vector-op masking is needed.
              DVE reduce_max per quarter directly from PSUM -> row maxes;
              DVE scalar_tensor_tensor extracts the positive-pair diagonal
              (accum-sum of P*I) -> pos.
              GPSIMD combines quarter maxes -> m, negm.
      pass 2: PE recomputes the quarters; ACT activation(Exp, bias=-m,
              accum_out) per quarter directly from PSUM = fused exp+rowsum.
              ACT: pe=exp(pos-m), lg=ln(sum); GPSIMD: loss = lg + m - pos.
  - Host: sum the 8 x [128,8] per-row losses, divide by 8192.

The toolchain's walrus codegen only allows ONE sync-wait per TPB instruction;
_split_waits() hoists extra waits onto injected NoOps post-Tile.
"""

import os
import numpy as np

N_CORES = 8
B = 4096
D = 128
NROWS = 2 * B           # 8192
ROWS_PER_CORE = NROWS // N_CORES       # 1024
TILES_PER_CORE = ROWS_PER_CORE // 128  # 8
QUARTER = 2048
TEMP = 0.07
MASK_NEG = -1.0e5

_cached = {}


def _split_waits(nc, limit=1):
    """Walrus here allows only `limit` sync-waits per instruction; hoist
    extras onto injected same-engine NoOps."""
    import bass_rust
    import concourse.mybir as mybir

    n = 0
    for f in nc.m.functions:
        for blk in f.blocks:
            new_insts = []
            for inst in blk.instructions:
                si = inst.sync_info
                waits = list(si.on_wait) if (si and si.on_wait) else []
                if len(waits) > limit:
                    for w in waits[:-limit]:
                        nop = bass_rust.InstNoOp(name=f"waitnop-{n}")
                        n += 1
                        nop.engine = inst.engine
                        nop.sync_info = mybir.SyncInfo(on_wait=[w], on_update=[])
                        new_insts.append(nop)
                    inst.sync_info = mybir.SyncInfo(
                        on_wait=waits[-limit:], on_update=list(si.on_update or [])
                    )
                new_insts.append(inst)
            blk.instructions = new_insts


def _build_module():
    import concourse.bass as bass
    import concourse.mybir as mybir
    from concourse.tile import TileContext
    from contextlib import ExitStack

    f32 = mybir.dt.float32
    Alu = mybir.AluOpType
    Act = mybir.ActivationFunctionType
    X = mybir.AxisListType.X

    nc = bass.Bass()

    zq_d = [
        nc.dram_tensor(f"zq{q}", [128, QUARTER], f32, kind="ExternalInput")
        for q in range(4)
    ]
    posi_d = nc.dram_tensor("posI", [128, 128], f32, kind="ExternalInput")
    mskb_d = nc.dram_tensor("mskB", [128, 128], f32, kind="ExternalInput")
    loss_d = nc.dram_tensor("loss", [128, TILES_PER_CORE], f32, kind="ExternalOutput")

    with ExitStack() as ctx:
        tc = ctx.enter_context(TileContext(nc))
        const = ctx.enter_context(tc.tile_pool(name="const", bufs=1))
        egp = ctx.enter_context(tc.tile_pool(name="egp", bufs=2))
        psum = ctx.enter_context(
            tc.tile_pool(name="psum", bufs=2, space=bass.MemorySpace.PSUM)
        )
        stats = ctx.enter_context(tc.tile_pool(name="stats", bufs=3))

        zqt = []
        for q in range(4):
            zt = const.tile([128, QUARTER], f32, tag=f"zq{q}")
            nc.gpsimd.dma_start(out=zt, in_=zq_d[q][:])
            zqt.append(zt)
        posit = const.tile([128, 128], f32, tag="posI")
        nc.gpsimd.dma_start(out=posit, in_=posi_d[:])
        mskbt = const.tile([128, 128], f32, tag="mskB")
        nc.gpsimd.dma_start(out=mskbt, in_=mskb_d[:])
        losst = const.tile([128, TILES_PER_CORE], f32, tag="losst")

        def quarter_matmuls(P, t, q, with_mask):
            lhsT = zqt[0][:, t * 128 : (t + 1) * 128]
            dj = (t * 128) // 512  # 512-chunk containing the diag block (q==0)
            for j in range(4):
                is_diag_chunk = with_mask and q == 0 and j == dj
                nc.tensor.matmul(
                    P[:, j * 512 : (j + 1) * 512],
                    lhsT,
                    zqt[q][:, j * 512 : (j + 1) * 512],
                    start=True,
                    stop=not is_diag_chunk,
                    skip_group_check=True,
                )
                if is_diag_chunk:
                    # self-diag block += -1e5*I  (I.T @ (-1e5*I) accumulate)
                    nc.tensor.matmul(
                        P[:, t * 128 : t * 128 + 128],
                        posit,
                        mskbt,
                        start=False,
                        stop=True,
                        skip_group_check=True,
                    )

        for t in range(TILES_PER_CORE):
            mx = stats.tile([128, 4], f32, tag="mx")
            pos = stats.tile([128, 1], f32, tag="pos")
            scr = stats.tile([128, 128], f32, tag="scr")

            # ---- pass 1: row max (+ pos) ----
            for q in range(4):
                P = psum.tile([128, QUARTER], f32, tag="P")
                quarter_matmuls(P, t, q, with_mask=True)
                if q == 2:
                    # positive-pair diag at q2-local cols [t*128, t*128+128)
                    nc.vector.scalar_tensor_tensor(
                        out=scr,
                        in0=P[:, t * 128 : t * 128 + 128],
                        scalar=1.0,
                        in1=posit,
                        op0=Alu.mult,
                        op1=Alu.mult,
                        accum_out=pos,
                    )
                nc.vector.reduce_max(out=mx[:, q : q + 1], in_=P, axis=X)

            negm = stats.tile([128, 1], f32, tag="negm")
            nc.vector.reduce_max(out=negm, in_=mx, axis=X, negate=True)

            # ---- pass 2: exp(x - m) row sums ----
            ssq = stats.tile([128, 4], f32, tag="ssq")
            for q in range(4):
                P2 = psum.tile([128, QUARTER], f32, tag="P")
                quarter_matmuls(P2, t, q, with_mask=True)
                eg = egp.tile([128, QUARTER], f32, tag="eg")
                nc.scalar.activation(
                    out=eg,
                    in_=P2,
                    func=Act.Exp,
                    bias=negm,
                    scale=1.0,
                    accum_out=ssq[:, q : q + 1],
                )

            pe = stats.tile([128, 1], f32, tag="pe")
            nc.scalar.activation(out=pe, in_=pos, func=Act.Exp, bias=negm, scale=1.0)
            s03 = stats.tile([128, 1], f32, tag="s03")
            stot = stats.tile([128, 1], f32, tag="stot")
            nc.vector.reduce_sum(out=s03, in_=ssq, axis=X)
            nc.vector.tensor_add(stot, s03, pe)

            # loss = ln(stot) + m - pos = lg - pos - negm
            lg = stats.tile([128, 1], f32, tag="lg")
            nc.scalar.activation(out=lg, in_=stot, func=Act.Ln)
            lp = stats.tile([128, 1], f32, tag="lp")
            nc.vector.tensor_sub(lp, lg, pos)
            nc.vector.tensor_sub(losst[:, t : t + 1], lp, negm)

        nc.gpsimd.dma_start(out=loss_d[:], in_=losst)

    _split_waits(nc)
    return nc


def _get_module():
    if "nc" not in _cached:
        _cached["nc"] = _build_module()
    return _cached["nc"]


def _host_inputs(z_i, z_j):
    z = np.concatenate(
        [np.asarray(z_i, np.float32), np.asarray(z_j, np.float32)], axis=0
    )
    s = np.float32(1.0 / np.sqrt(TEMP))
    zT = np.ascontiguousarray((z * s).T)  # [128, 8192]

    posI = np.eye(128, dtype=np.float32)
    mskB = np.float32(MASK_NEG) * np.eye(128, dtype=np.float32)

    in_maps = []
    for c in range(N_CORES):
        k = c * ROWS_PER_CORE
        rot = np.concatenate([zT[:, k:], zT[:, :k]], axis=1)
        im = {
            f"zq{q}": np.ascontiguousarray(rot[:, q * QUARTER : (q + 1) * QUARTER])
            for q in range(4)
        }
        im["posI"] = posI
        im["mskB"] = mskB
        in_maps.append(im)
    return in_maps


def run_full(z_i, z_j, trace=False, trace_kwargs=None):
    """Run on 8 cores; returns (loss_scalar, BassKernelResults)."""
    from concourse.bass_utils import run_bass_kernel_spmd

    nc = _get_module()
    in_maps = _host_inputs(z_i, z_j)
    res = run_bass_kernel_spmd(
        nc,
        in_maps,
        core_ids=list(range(N_CORES)),
        trace=trace,
        **(trace_kwargs or {}),
    )
    total = np.float64(0.0)
    for c in range(N_CORES):
        total += res.results[c]["loss"].astype(np.float64).sum()
    loss = np.array(total / NROWS, dtype=np.float32)
    return loss, res


def kernel(z_i, z_j):
    loss, _ = run_full(z_i, z_j, trace=bool(os.environ.get("KERNEL_TRACE")))
    return loss


# revision 7
# speedup vs baseline: 1.5301x; 1.5301x over previous
"""NT-Xent loss (B=4096, D=128, T=0.07) on 8 Trainium2 NeuronCores.

Strategy (one SPMD Bass program, 8 cores):
  - Host: z = concat(z_i, z_j) [8192,128], scale by 1/sqrt(T), transpose to
    zT [128, 8192].  Core c receives zT rotated left by c*1024 columns so its
    own 1024 rows sit at columns 0..1023 -> the self-sim diagonal block and
    the positive-pair diagonal block land at compile-time-constant column
    offsets on every core (one uniform SPMD program, no partition-id logic).
  - Device, per 128-row tile t (8 tiles per core), two passes over the same
    [128, 8192] similarity slab (PE matmul is cheap; PSUM egress is not):
      pass 1: PE 17 matmuls -> PSUM quarters [128,2048]; the self-diag block
              gets -1e5*I added via an extra accumulating matmul (identity x
              identity trick) so no vector-op masking is needed.
              DVE reduce_max per quarter directly from PSUM -> row maxes;
              DVE scalar_tensor_tensor extracts the positive-pair diagonal
              (accum-sum of P*I) -> pos.
              GPSIMD combines quarter maxes -> m, negm.
      pass 2: PE recomputes the quarters; ACT activation(Exp, bias=-m,
              accum_out) per quarter directly from PSUM = fused exp+rowsum.
              ACT: pe=exp(pos-m), lg=ln(sum); GPSIMD: loss = lg + m - pos.
  - Host: sum the 8 x [128,8] per-row losses, divide by 8192.

The toolchain's walrus codegen only allows ONE sync-wait per TPB instruction;
_split_waits() hoists extra waits onto injected NoOps post-Tile.
"""

import os
import numpy as np

N_CORES = 8
B = 4096
D = 128
NROWS = 2 * B           # 8192
ROWS_PER_CORE = NROWS // N_CORES       # 1024
TILES_PER_CORE = ROWS_PER_CORE // 128  # 8
QUARTER = 2048
TEMP = 0.07
MASK_NEG = -1.0e5

_cached = {}


def _split_waits(nc, limit=1):
    """Walrus here allows only `limit` sync-waits per instruction; hoist
    extras onto injected same-engine NoOps."""
    import bass_rust
    import concourse.mybir as mybir

    n = 0
    for f in nc.m.functions:
        for blk in f.blocks:
            new_insts = []
            for inst in blk.instructions:
                si = inst.sync_info
                waits = list(si.on_wait) if (si and si.on_wait) else []
                if len(waits) > limit:
                    for w in waits[:-limit]:
                        nop = bass_rust.InstNoOp(name=f"waitnop-{n}")
                        n += 1
                        nop.engine = inst.engine
                        nop.sync_info = mybir.SyncInfo(on_wait=[w], on_update=[])
                        new_insts.append(nop)
                    inst.sync_info = mybir.SyncInfo(
                        on_wait=waits[-limit:], on_update=list(si.on_update or [])
                    )
                new_insts.append(inst)
            blk.instructions = new_insts


def _build_module():
    import concourse.bass as bass
    import concourse.mybir as mybir
    from concourse.tile import TileContext
    from contextlib import ExitStack

    f32 = mybir.dt.float32
    f16 = mybir.dt.float16
    Alu = mybir.AluOpType
    Act = mybir.ActivationFunctionType
    X = mybir.AxisListType.X

    nc = bass.Bass()

    zq_d = [
        nc.dram_tensor(f"zq{q}", [128, QUARTER], f16, kind="ExternalInput")
        for q in range(4)
    ]
    posi_d = nc.dram_tensor("posI", [128, 128], f32, kind="ExternalInput")
    mskb_d = nc.dram_tensor("mskB", [128, 128], f32, kind="ExternalInput")
    loss_d = nc.dram_tensor("loss", [128, TILES_PER_CORE], f32, kind="ExternalOutput")

    with ExitStack() as ctx:
        tc = ctx.enter_context(TileContext(nc))
        const = ctx.enter_context(tc.tile_pool(name="const", bufs=1))
        egp = ctx.enter_context(tc.tile_pool(name="egp", bufs=2))
        psum = ctx.enter_context(
            tc.tile_pool(name="psum", bufs=2, space=bass.MemorySpace.PSUM)
        )
        stats = ctx.enter_context(tc.tile_pool(name="stats", bufs=3))

        zqt = []
        for q in range(4):
            zt = const.tile([128, QUARTER], f16, tag=f"zq{q}")
            nc.gpsimd.dma_start(out=zt, in_=zq_d[q][:])
            zqt.append(zt)
        posit = const.tile([128, 128], f32, tag="posI")
        nc.gpsimd.dma_start(out=posit, in_=posi_d[:])
        mskbt = const.tile([128, 128], f32, tag="mskB")
        nc.gpsimd.dma_start(out=mskbt, in_=mskb_d[:])
        losst = const.tile([128, TILES_PER_CORE], f32, tag="losst")

        def quarter_matmuls(P, t, q, with_mask):
            lhsT = zqt[0][:, t * 128 : (t + 1) * 128]
            dj = (t * 128) // 512  # 512-chunk containing the diag block (q==0)
            for j in range(4):
                is_diag_chunk = with_mask and q == 0 and j == dj
                nc.tensor.matmul(
                    P[:, j * 512 : (j + 1) * 512],
                    lhsT,
                    zqt[q][:, j * 512 : (j + 1) * 512],
                    start=True,
                    stop=not is_diag_chunk,
                    skip_group_check=True,
                )
                if is_diag_chunk:
                    # self-diag block += -1e5*I  (I.T @ (-1e5*I) accumulate)
                    nc.tensor.matmul(
                        P[:, t * 128 : t * 128 + 128],
                        posit,
                        mskbt,
                        start=False,
                        stop=True,
                        skip_group_check=True,
                    )

        for t in range(TILES_PER_CORE):
            mx = stats.tile([128, 4], f32, tag="mx")
            pos = stats.tile([128, 1], f32, tag="pos")
            scr = stats.tile([128, 128], f32, tag="scr")

            # ---- pass 1: row max (+ pos) ----
            for q in range(4):
                P = psum.tile([128, QUARTER], f32, tag="P")
                quarter_matmuls(P, t, q, with_mask=True)
                if q == 2:
                    # positive-pair diag at q2-local cols [t*128, t*128+128)
                    nc.vector.scalar_tensor_tensor(
                        out=scr,
                        in0=P[:, t * 128 : t * 128 + 128],
                        scalar=1.0,
                        in1=posit,
                        op0=Alu.mult,
                        op1=Alu.mult,
                        accum_out=pos,
                    )
                nc.vector.reduce_max(out=mx[:, q : q + 1], in_=P, axis=X)

            negm = stats.tile([128, 1], f32, tag="negm")
            nc.vector.reduce_max(out=negm, in_=mx, axis=X, negate=True)

            # ---- pass 2: exp(x - m) row sums ----
            ssq = stats.tile([128, 4], f32, tag="ssq")
            for q in range(4):
                P2 = psum.tile([128, QUARTER], f32, tag="P")
                quarter_matmuls(P2, t, q, with_mask=True)
                eg = egp.tile([128, QUARTER], f32, tag="eg")
                nc.scalar.activation(
                    out=eg,
                    in_=P2,
                    func=Act.Exp,
                    bias=negm,
                    scale=1.0,
                    accum_out=ssq[:, q : q + 1],
                )

            pe = stats.tile([128, 1], f32, tag="pe")
            nc.scalar.activation(out=pe, in_=pos, func=Act.Exp, bias=negm, scale=1.0)
            s03 = stats.tile([128, 1], f32, tag="s03")
            stot = stats.tile([128, 1], f32, tag="stot")
            nc.vector.reduce_sum(out=s03, in_=ssq, axis=X)
            nc.vector.tensor_add(stot, s03, pe)

            # loss = ln(stot) + m - pos = lg - pos - negm
            lg = stats.tile([128, 1], f32, tag="lg")
            nc.scalar.activation(out=lg, in_=stot, func=Act.Ln)
            lp = stats.tile([128, 1], f32, tag="lp")
            nc.vector.tensor_sub(lp, lg, pos)
            nc.vector.tensor_sub(losst[:, t : t + 1], lp, negm)

        nc.gpsimd.dma_start(out=loss_d[:], in_=losst)

    _split_waits(nc)
    return nc


def _get_module():
    if "nc" not in _cached:
        _cached["nc"] = _build_module()
    return _cached["nc"]


def _host_inputs(z_i, z_j):
    z = np.concatenate(
        [np.asarray(z_i, np.float32), np.asarray(z_j, np.float32)], axis=0
    )
    s = np.float32(1.0 / np.sqrt(TEMP))
    zT = np.ascontiguousarray((z * s).T).astype(np.float16)  # [128, 8192]

    posI = np.eye(128, dtype=np.float32)
    mskB = np.float32(MASK_NEG) * np.eye(128, dtype=np.float32)

    in_maps = []
    for c in range(N_CORES):
        k = c * ROWS_PER_CORE
        rot = np.concatenate([zT[:, k:], zT[:, :k]], axis=1)
        im = {
            f"zq{q}": np.ascontiguousarray(rot[:, q * QUARTER : (q + 1) * QUARTER])
            for q in range(4)
        }
        im["posI"] = posI
        im["mskB"] = mskB
        in_maps.append(im)
    return in_maps


def run_full(z_i, z_j, trace=False, trace_kwargs=None):
    """Run on 8 cores; returns (loss_scalar, BassKernelResults)."""
    from concourse.bass_utils import run_bass_kernel_spmd

    nc = _get_module()
    in_maps = _host_inputs(z_i, z_j)
    res = run_bass_kernel_spmd(
        nc,
        in_maps,
        core_ids=list(range(N_CORES)),
        trace=trace,
        **(trace_kwargs or {}),
    )
    total = np.float64(0.0)
    for c in range(N_CORES):
        total += res.results[c]["loss"].astype(np.float64).sum()
    loss = np.array(total / NROWS, dtype=np.float32)
    return loss, res


def kernel(z_i, z_j):
    loss, _ = run_full(z_i, z_j, trace=bool(os.environ.get("KERNEL_TRACE")))
    return loss


# revision 9
# speedup vs baseline: 1.6525x; 1.0800x over previous
"""NT-Xent loss (B=4096, D=128, T=0.07) on 8 Trainium2 NeuronCores.

Strategy (one SPMD Bass program, 8 cores):
  - Host: z = concat(z_i, z_j) [8192,128], scale by 1/sqrt(T), transpose to
    zT [128, 8192], cast fp16 (PE runs fp16 at 4x the fp32 rate; validated
    loss rel-err ~1.4e-6).  Core c receives zT rotated left by c*1024 cols so
    its own 1024 rows sit at columns 0..1023 -> the self-sim diag block and
    the positive-pair diag block land at compile-time-constant offsets on
    every core (one uniform SPMD program).
  - Device, per 128-row tile t (8 tiles/core), two passes over the same
    [128, 8192] similarity slab (PE matmul is cheap; PSUM egress is not):
      pass 1: PE matmuls -> PSUM quarters [128,2048]; self-diag block gets
              -1e5*I added via an extra accumulating matmul (identity trick).
              DVE reduce_max per quarter straight from PSUM; DVE
              scalar_tensor_tensor extracts the positive-pair diagonal.
              For ACT_EVAC quarters, ACT copies PSUM->SBUF instead and DVE
              reduces from SBUF at 2x (engine balancing).
      pass 2: PE recomputes the quarters; ACT activation(Exp, bias=-m,
              accum_out) per quarter straight from PSUM = fused exp+rowsum.
              ACT_EVAC quarters exp from their SBUF copy (no recompute).
    The two passes of ADJACENT tiles are interleaved quarter-by-quarter so
    DVE (pass 1 of tile t) and ACT (pass 2 of tile t-1) run concurrently
    instead of alternating.
  - Host: sum the 8 x [128,8] per-row losses, divide by 8192.

This toolchain's walrus allows only ONE sync-wait per TPB instruction;
_split_waits() hoists extra waits onto injected NoOps post-Tile.
"""

import os
import numpy as np

N_CORES = 8
B = 4096
NROWS = 2 * B           # 8192
ROWS_PER_CORE = NROWS // N_CORES       # 1024
TILES_PER_CORE = ROWS_PER_CORE // 128  # 8
QUARTER = 2048
TEMP = 0.07
MASK_NEG = -1.0e5

# tiles whose quarter-3 takes the ACT-evac path (engine load balancing)
ACT_EVAC_TILES = frozenset({0, 2, 3, 5, 7})

_cached = {}


def _split_waits(nc, limit=1):
    import bass_rust
    import concourse.mybir as mybir

    n = 0
    for f in nc.m.functions:
        for blk in f.blocks:
            new_insts = []
            for inst in blk.instructions:
                si = inst.sync_info
                waits = list(si.on_wait) if (si and si.on_wait) else []
                if len(waits) > limit:
                    for w in waits[:-limit]:
                        nop = bass_rust.InstNoOp(name=f"waitnop-{n}")
                        n += 1
                        nop.engine = inst.engine
                        nop.sync_info = mybir.SyncInfo(on_wait=[w], on_update=[])
                        new_insts.append(nop)
                    inst.sync_info = mybir.SyncInfo(
                        on_wait=waits[-limit:], on_update=list(si.on_update or [])
                    )
                new_insts.append(inst)
            blk.instructions = new_insts


def _build_module():
    import concourse.bass as bass
    import concourse.mybir as mybir
    from concourse.tile import TileContext
    from contextlib import ExitStack

    f32 = mybir.dt.float32
    f16 = mybir.dt.float16
    Alu = mybir.AluOpType
    Act = mybir.ActivationFunctionType
    X = mybir.AxisListType.X

    nc = bass.Bass()

    zq_d = [
        nc.dram_tensor(f"zq{q}", [128, QUARTER], f16, kind="ExternalInput")
        for q in range(4)
    ]
    posi_d = nc.dram_tensor("posI", [128, 128], f32, kind="ExternalInput")
    mskb_d = nc.dram_tensor("mskB", [128, 128], f32, kind="ExternalInput")
    loss_d = nc.dram_tensor("loss", [128, TILES_PER_CORE], f32, kind="ExternalOutput")

    with ExitStack() as ctx:
        tc = ctx.enter_context(TileContext(nc))
        const = ctx.enter_context(tc.tile_pool(name="const", bufs=1))
        egp = ctx.enter_context(tc.tile_pool(name="egp", bufs=2))
        simp = ctx.enter_context(tc.tile_pool(name="simp", bufs=2))
        psum = ctx.enter_context(
            tc.tile_pool(name="psum", bufs=2, space=bass.MemorySpace.PSUM)
        )
        stats = ctx.enter_context(tc.tile_pool(name="stats", bufs=3))

        zqt = []
        for q in range(4):
            zt = const.tile([128, QUARTER], f16, tag=f"zq{q}")
            nc.gpsimd.dma_start(out=zt, in_=zq_d[q][:])
            zqt.append(zt)
        posit = const.tile([128, 128], f32, tag="posI")
        nc.gpsimd.dma_start(out=posit, in_=posi_d[:])
        mskbt = const.tile([128, 128], f32, tag="mskB")
        nc.gpsimd.dma_start(out=mskbt, in_=mskb_d[:])
        losst = const.tile([128, TILES_PER_CORE], f32, tag="losst")

        def quarter_matmuls(P, t, q):
            lhsT = zqt[0][:, t * 128 : (t + 1) * 128]
            dj = (t * 128) // 512  # 512-chunk containing the self-diag (q==0)
            for j in range(4):
                is_diag_chunk = q == 0 and j == dj
                nc.tensor.matmul(
                    P[:, j * 512 : (j + 1) * 512],
                    lhsT,
                    zqt[q][:, j * 512 : (j + 1) * 512],
                    start=True,
                    stop=not is_diag_chunk,
                    skip_group_check=True,
                )
                if is_diag_chunk:
                    # self-diag block += -1e5*I  (I.T @ (-1e5*I) accumulated)
                    nc.tensor.matmul(
                        P[:, t * 128 : t * 128 + 128],
                        posit,
                        mskbt,
                        start=False,
                        stop=True,
                        skip_group_check=True,
                    )

        # per-tile state carried across the interleaved pipeline
        state = {}

        def pass1_quarter(t, q):
            st = state[t]
            if q == 3 and t in ACT_EVAC_TILES:
                P = psum.tile([128, QUARTER], f32, tag="P")
                quarter_matmuls(P, t, q)
                sim3 = simp.tile([128, QUARTER], f32, tag="sim3")
                nc.scalar.copy(sim3, P)  # ACT evacuates; DVE reduces at 2x
                nc.vector.reduce_max(out=st["mx"][:, 3:4], in_=sim3, axis=X)
                st["sim3"] = sim3
            else:
                P = psum.tile([128, QUARTER], f32, tag="P")
                quarter_matmuls(P, t, q)
                if q == 2:
                    nc.vector.scalar_tensor_tensor(
                        out=st["scr"],
                        in0=P[:, t * 128 : t * 128 + 128],
                        scalar=1.0,
                        in1=posit,
                        op0=Alu.mult,
                        op1=Alu.mult,
                        accum_out=st["pos"],
                    )
                nc.vector.reduce_max(out=st["mx"][:, q : q + 1], in_=P, axis=X)

        def pass1_negm(t):
            st = state[t]
            nc.vector.reduce_max(out=st["negm"], in_=st["mx"], axis=X, negate=True)

        def pass2_quarter(t, q):
            st = state[t]
            if q == 3 and t in ACT_EVAC_TILES:
                eg = egp.tile([128, QUARTER], f32, tag="eg")
                nc.scalar.activation(
                    out=eg, in_=st["sim3"], func=Act.Exp, bias=st["negm"],
                    scale=1.0, accum_out=st["ssq"][:, 3:4],
                )
            else:
                P2 = psum.tile([128, QUARTER], f32, tag="P")
                quarter_matmuls(P2, t, q)
                eg = egp.tile([128, QUARTER], f32, tag="eg")
                nc.scalar.activation(
                    out=eg, in_=P2, func=Act.Exp, bias=st["negm"],
                    scale=1.0, accum_out=st["ssq"][:, q : q + 1],
                )

        def finish_tile(t):
            st = state[t]
            pe = stats.tile([128, 1], f32, tag="pe")
            nc.scalar.activation(
                out=pe, in_=st["pos"], func=Act.Exp, bias=st["negm"], scale=1.0
            )
            s03 = stats.tile([128, 1], f32, tag="s03")
            stot = stats.tile([128, 1], f32, tag="stot")
            nc.vector.reduce_sum(out=s03, in_=st["ssq"], axis=X)
            nc.vector.tensor_add(stot, s03, pe)
            # loss = ln(stot) + m - pos = lg - pos - negm
            lg = stats.tile([128, 1], f32, tag="lg")
            nc.scalar.activation(out=lg, in_=stot, func=Act.Ln)
            lp = stats.tile([128, 1], f32, tag="lp")
            nc.vector.tensor_sub(lp, lg, st["pos"])
            nc.vector.tensor_sub(losst[:, t : t + 1], lp, st["negm"])
            del state[t]

        def new_tile_state(t):
            mx = stats.tile([128, 4], f32, tag="mx")
            pos = stats.tile([128, 1], f32, tag="pos")
            scr = stats.tile([128, 128], f32, tag="scr")
            negm = stats.tile([128, 1], f32, tag="negm")
            ssq = stats.tile([128, 4], f32, tag="ssq")
            state[t] = {"mx": mx, "pos": pos, "scr": scr, "negm": negm, "ssq": ssq}

        # software pipeline: pass2(t-1) and pass1(t) interleaved per quarter
        new_tile_state(0)
        for q in range(4):
            pass1_quarter(0, q)
        pass1_negm(0)
        for t in range(1, TILES_PER_CORE):
            new_tile_state(t)
            for q in range(4):
                pass2_quarter(t - 1, q)
                pass1_quarter(t, q)
            pass1_negm(t)
            finish_tile(t - 1)
        for q in range(4):
            pass2_quarter(TILES_PER_CORE - 1, q)
        finish_tile(TILES_PER_CORE - 1)

        nc.gpsimd.dma_start(out=loss_d[:], in_=losst)

    _split_waits(nc)
    return nc


def _get_module():
    if "nc" not in _cached:
        _cached["nc"] = _build_module()
    return _cached["nc"]


def _host_inputs(z_i, z_j):
    z = np.concatenate(
        [np.asarray(z_i, np.float32), np.asarray(z_j, np.float32)], axis=0
    )
    s = np.float32(1.0 / np.sqrt(TEMP))
    zT = np.ascontiguousarray((z * s).T).astype(np.float16)  # [128, 8192]

    posI = np.eye(128, dtype=np.float32)
    mskB = np.float32(MASK_NEG) * np.eye(128, dtype=np.float32)

    in_maps = []
    for c in range(N_CORES):
        k = c * ROWS_PER_CORE
        rot = np.concatenate([zT[:, k:], zT[:, :k]], axis=1)
        im = {
            f"zq{q}": np.ascontiguousarray(rot[:, q * QUARTER : (q + 1) * QUARTER])
            for q in range(4)
        }
        im["posI"] = posI
        im["mskB"] = mskB
        in_maps.append(im)
    return in_maps


def run_full(z_i, z_j, trace=False, trace_kwargs=None):
    """Run on 8 cores; returns (loss_scalar, BassKernelResults)."""
    from concourse.bass_utils import run_bass_kernel_spmd

    nc = _get_module()
    in_maps = _host_inputs(z_i, z_j)
    res = run_bass_kernel_spmd(
        nc,
        in_maps,
        core_ids=list(range(N_CORES)),
        trace=trace,
        **(trace_kwargs or {}),
    )
    total = np.float64(0.0)
    for c in range(N_CORES):
        total += res.results[c]["loss"].astype(np.float64).sum()
    loss = np.array(total / NROWS, dtype=np.float32)
    return loss, res


def kernel(z_i, z_j):
    loss, _ = run_full(z_i, z_j, trace=bool(os.environ.get("KERNEL_TRACE")))
    return loss


# revision 10
# speedup vs baseline: 1.8918x; 1.1448x over previous
"""NT-Xent loss (B=4096, D=128, T=0.07) on 8 Trainium2 NeuronCores.

Strategy (one SPMD Bass program, 8 cores):
  - Host: z = concat(z_i, z_j) [8192,128], scale by 1/sqrt(T), transpose to
    zT [128, 8192], cast fp16 (PE runs fp16 at 4x the fp32 rate; validated
    loss rel-err ~1.4e-6).  Core c receives zT rotated left by c*1024 cols so
    its own 1024 rows sit at columns 0..1023 -> the self-sim diag block and
    the positive-pair diag block land at compile-time-constant offsets on
    every core (one uniform SPMD program).
  - Device, per 128-row tile t (8 tiles/core), ONE pass over the [128, 8192]
    similarity slab in four [128,2048] PSUM quarters:
      PE   : 4 matmuls (N=512, fp16) per quarter; the self-diag block gets
             -1e5*I added via an extra accumulating matmul (identity trick),
             so it can never win the max and exp() flushes it to 0.
      DVE  : reduce_max(negate) straight from PSUM -> per-quarter -max m_q;
             scalar_tensor_tensor extracts the positive-pair diagonal.
      ACT  : activation(Exp, bias=-m_q, accum_out) straight from the same
             PSUM quarter = fused exp + row-sum with a PER-QUARTER shift.
             (DVE and ACT read the same quarter concurrently via separate
             PSUM ports; PE fills the other buffer meanwhile.)
      tail : quarter sums are rescaled exactly: stot = sum_q ssq_q*e^{m_q-m}
             (+ e^{pos-m} for the duplicated positive), loss = ln(stot)+m-pos.
             All tail ops are [128,4]/[128,1] sized.
  - Host: sum the 8 x [128,8] per-row losses, divide by 8192.

This avoids any second PE pass and any PSUM->SBUF evacuation of the slab:
each PSUM element is read exactly twice (once by DVE for the max, once by
ACT for the exp-sum), which is the minimum this algorithm needs.

The toolchain's walrus allows only ONE sync-wait per TPB instruction;
_split_waits() hoists extra waits onto injected NoOps post-Tile.
"""

import os
import numpy as np

N_CORES = 8
B = 4096
NROWS = 2 * B           # 8192
ROWS_PER_CORE = NROWS // N_CORES       # 1024
TILES_PER_CORE = ROWS_PER_CORE // 128  # 8
QUARTER = 2048
TEMP = 0.07
MASK_NEG = -1.0e5

_cached = {}


def _split_waits(nc, limit=1):
    import bass_rust
    import concourse.mybir as mybir

    n = 0
    for f in nc.m.functions:
        for blk in f.blocks:
            new_insts = []
            for inst in blk.instructions:
                si = inst.sync_info
                waits = list(si.on_wait) if (si and si.on_wait) else []
                if len(waits) > limit:
                    for w in waits[:-limit]:
                        nop = bass_rust.InstNoOp(name=f"waitnop-{n}")
                        n += 1
                        nop.engine = inst.engine
                        nop.sync_info = mybir.SyncInfo(on_wait=[w], on_update=[])
                        new_insts.append(nop)
                    inst.sync_info = mybir.SyncInfo(
                        on_wait=waits[-limit:], on_update=list(si.on_update or [])
                    )
                new_insts.append(inst)
            blk.instructions = new_insts


def _build_module():
    import concourse.bass as bass
    import concourse.mybir as mybir
    from concourse.tile import TileContext
    from contextlib import ExitStack

    f32 = mybir.dt.float32
    f16 = mybir.dt.float16
    Alu = mybir.AluOpType
    Act = mybir.ActivationFunctionType
    X = mybir.AxisListType.X

    nc = bass.Bass()

    zq_d = [
        nc.dram_tensor(f"zq{q}", [128, QUARTER], f16, kind="ExternalInput")
        for q in range(4)
    ]
    posi_d = nc.dram_tensor("posI", [128, 128], f32, kind="ExternalInput")
    mskb_d = nc.dram_tensor("mskB", [128, 128], f32, kind="ExternalInput")
    loss_d = nc.dram_tensor("loss", [128, TILES_PER_CORE], f32, kind="ExternalOutput")

    with ExitStack() as ctx:
        tc = ctx.enter_context(TileContext(nc))
        const = ctx.enter_context(tc.tile_pool(name="const", bufs=1))
        egp = ctx.enter_context(tc.tile_pool(name="egp", bufs=2))
        psum = ctx.enter_context(
            tc.tile_pool(name="psum", bufs=2, space=bass.MemorySpace.PSUM)
        )
        stats = ctx.enter_context(tc.tile_pool(name="stats", bufs=3))

        zqt = []
        for q in range(4):
            zt = const.tile([128, QUARTER], f16, tag=f"zq{q}")
            nc.gpsimd.dma_start(out=zt, in_=zq_d[q][:])
            zqt.append(zt)
        posit = const.tile([128, 128], f32, tag="posI")
        nc.gpsimd.dma_start(out=posit, in_=posi_d[:])
        mskbt = const.tile([128, 128], f32, tag="mskB")
        nc.gpsimd.dma_start(out=mskbt, in_=mskb_d[:])
        losst = const.tile([128, TILES_PER_CORE], f32, tag="losst")

        def quarter_matmuls(P, t, q):
            lhsT = zqt[0][:, t * 128 : (t + 1) * 128]
            dj = (t * 128) // 512  # 512-chunk containing the self-diag (q==0)
            for j in range(4):
                is_diag_chunk = q == 0 and j == dj
                nc.tensor.matmul(
                    P[:, j * 512 : (j + 1) * 512],
                    lhsT,
                    zqt[q][:, j * 512 : (j + 1) * 512],
                    start=True,
                    stop=not is_diag_chunk,
                    skip_group_check=True,
                )
                if is_diag_chunk:
                    # self-diag block += -1e5*I  (I.T @ (-1e5*I) accumulated)
                    nc.tensor.matmul(
                        P[:, t * 128 : t * 128 + 128],
                        posit,
                        mskbt,
                        start=False,
                        stop=True,
                        skip_group_check=True,
                    )

        for t in range(TILES_PER_CORE):
            nm = stats.tile([128, 4], f32, tag="nm")     # -m_q per quarter
            ssq = stats.tile([128, 4], f32, tag="ssq")   # sum e^{x-m_q}
            pos = stats.tile([128, 1], f32, tag="pos")
            scr = stats.tile([128, 128], f32, tag="scr")

            for q in range(4):
                P = psum.tile([128, QUARTER], f32, tag="P")
                quarter_matmuls(P, t, q)
                if q == 2:
                    # positive-pair diag at q2-local cols [t*128, t*128+128)
                    nc.vector.scalar_tensor_tensor(
                        out=scr,
                        in0=P[:, t * 128 : t * 128 + 128],
                        scalar=1.0,
                        in1=posit,
                        op0=Alu.mult,
                        op1=Alu.mult,
                        accum_out=pos,
                    )
                nc.vector.reduce_max(out=nm[:, q : q + 1], in_=P, axis=X, negate=True)
                eg = egp.tile([128, QUARTER], f32, tag="eg")
                nc.scalar.activation(
                    out=eg,
                    in_=P,
                    func=Act.Exp,
                    bias=nm[:, q : q + 1],
                    scale=1.0,
                    accum_out=ssq[:, q : q + 1],
                )

            # tail: exact recombination of the four quarter-shifted sums
            # m = global row max = -min_q nm_q  (mt := -m)
            mt = stats.tile([128, 1], f32, tag="mt")
            nc.vector.tensor_reduce(out=mt, in_=nm, axis=X, op=Alu.min)
            d = stats.tile([128, 4], f32, tag="d")
            nc.vector.tensor_scalar(
                out=d, in0=nm, scalar1=mt, scalar2=None, op0=Alu.subtract
            )
            f = stats.tile([128, 4], f32, tag="f")
            nc.scalar.activation(out=f, in_=d, func=Act.Exp, scale=-1.0)
            s03 = stats.tile([128, 1], f32, tag="s03")
            nc.vector.scalar_tensor_tensor(
                out=scr[:, 0:4],
                in0=ssq,
                scalar=1.0,
                in1=f,
                op0=Alu.mult,
                op1=Alu.mult,
                accum_out=s03,
            )
            pe = stats.tile([128, 1], f32, tag="pe")   # e^{pos - m}
            nc.scalar.activation(out=pe, in_=pos, func=Act.Exp, bias=mt, scale=1.0)
            stot = stats.tile([128, 1], f32, tag="stot")
            nc.vector.tensor_add(stot, s03, pe)
            # loss = ln(stot) + m - pos = lg - mt - pos
            lg = stats.tile([128, 1], f32, tag="lg")
            nc.scalar.activation(out=lg, in_=stot, func=Act.Ln)
            lp = stats.tile([128, 1], f32, tag="lp")
            nc.vector.tensor_sub(lp, lg, mt)
            nc.vector.tensor_sub(losst[:, t : t + 1], lp, pos)

        nc.gpsimd.dma_start(out=loss_d[:], in_=losst)

    _split_waits(nc)
    return nc


def _get_module():
    if "nc" not in _cached:
        _cached["nc"] = _build_module()
    return _cached["nc"]


def _host_inputs(z_i, z_j):
    z = np.concatenate(
        [np.asarray(z_i, np.float32), np.asarray(z_j, np.float32)], axis=0
    )
    s = np.float32(1.0 / np.sqrt(TEMP))
    zT = np.ascontiguousarray((z * s).T).astype(np.float16)  # [128, 8192]

    posI = np.eye(128, dtype=np.float32)
    mskB = np.float32(MASK_NEG) * np.eye(128, dtype=np.float32)

    in_maps = []
    for c in range(N_CORES):
        k = c * ROWS_PER_CORE
        rot = np.concatenate([zT[:, k:], zT[:, :k]], axis=1)
        im = {
            f"zq{q}": np.ascontiguousarray(rot[:, q * QUARTER : (q + 1) * QUARTER])
            for q in range(4)
        }
        im["posI"] = posI
        im["mskB"] = mskB
        in_maps.append(im)
    return in_maps


def run_full(z_i, z_j, trace=False, trace_kwargs=None):
    """Run on 8 cores; returns (loss_scalar, BassKernelResults)."""
    from concourse.bass_utils import run_bass_kernel_spmd

    nc = _get_module()
    in_maps = _host_inputs(z_i, z_j)
    res = run_bass_kernel_spmd(
        nc,
        in_maps,
        core_ids=list(range(N_CORES)),
        trace=trace,
        **(trace_kwargs or {}),
    )
    total = np.float64(0.0)
    for c in range(N_CORES):
        total += res.results[c]["loss"].astype(np.float64).sum()
    loss = np.array(total / NROWS, dtype=np.float32)
    return loss, res


def kernel(z_i, z_j):
    loss, _ = run_full(z_i, z_j, trace=bool(os.environ.get("KERNEL_TRACE")))
    return loss


# revision 11
# speedup vs baseline: 2.3876x; 1.2621x over previous
"""NT-Xent loss (B=4096, D=128, T=0.07) on 8 Trainium2 NeuronCores.

Strategy (one SPMD Bass program, 8 cores):
  - Host: z = concat(z_i, z_j) [8192,128], scale by 1/sqrt(T), transpose to
    zT [128, 8192], cast fp16 (PE runs fp16 at 4x the fp32 rate; validated
    loss rel-err ~1.4e-6).  Core c receives zT rotated left by c*1024 cols so
    its own 1024 rows sit at columns 0..1023 -> the self-sim diag block and
    the positive-pair diag block land at compile-time-constant offsets on
    every core (one uniform SPMD program).
  - Device, per 128-row tile t (8 tiles/core), ONE pass over the [128, 8192]
    similarity slab in eight [128,1024] PSUM chunks (4 PSUM buffers in flight so the
    fill->reduce->exp->release chain pipelines):
      PE   : 2 matmuls (N=512, fp16) per chunk; the self-diag block gets
             -1e5*I added via an extra accumulating matmul (identity trick),
             so it can never win the max and exp() flushes it to 0.
      DVE  : reduce_max(negate) straight from PSUM -> per-quarter -max m_q;
             scalar_tensor_tensor extracts the positive-pair diagonal.
      ACT  : activation(Exp, bias=-m_q, accum_out) straight from the same
             PSUM quarter = fused exp + row-sum with a PER-QUARTER shift.
             (DVE and ACT read the same quarter concurrently via separate
             PSUM ports; PE fills the other buffer meanwhile.)
      tail : quarter sums are rescaled exactly: stot = sum_q ssq_q*e^{m_q-m}
             (+ e^{pos-m} for the duplicated positive), loss = ln(stot)+m-pos.
             All tail ops are [128,4]/[128,1] sized.
  - Host: sum the 8 x [128,8] per-row losses, divide by 8192.

This avoids any second PE pass and any PSUM->SBUF evacuation of the slab:
each PSUM element is read exactly twice (once by DVE for the max, once by
ACT for the exp-sum), which is the minimum this algorithm needs.

The toolchain's walrus allows only ONE sync-wait per TPB instruction;
_split_waits() hoists extra waits onto injected NoOps post-Tile.
"""

import os
import numpy as np

N_CORES = 8
B = 4096
NROWS = 2 * B           # 8192
ROWS_PER_CORE = NROWS // N_CORES       # 1024
TILES_PER_CORE = ROWS_PER_CORE // 128  # 8
CHUNK = 1024
NCHUNK = 8192 // CHUNK  # 8
TEMP = 0.07
MASK_NEG = -1.0e5

_cached = {}


def _split_waits(nc, limit=1):
    import bass_rust
    import concourse.mybir as mybir

    n = 0
    for f in nc.m.functions:
        for blk in f.blocks:
            new_insts = []
            for inst in blk.instructions:
                si = inst.sync_info
                waits = list(si.on_wait) if (si and si.on_wait) else []
                if len(waits) > limit:
                    for w in waits[:-limit]:
                        nop = bass_rust.InstNoOp(name=f"waitnop-{n}")
                        n += 1
                        nop.engine = inst.engine
                        nop.sync_info = mybir.SyncInfo(on_wait=[w], on_update=[])
                        new_insts.append(nop)
                    inst.sync_info = mybir.SyncInfo(
                        on_wait=waits[-limit:], on_update=list(si.on_update or [])
                    )
                new_insts.append(inst)
            blk.instructions = new_insts


def _build_module():
    import concourse.bass as bass
    import concourse.mybir as mybir
    from concourse.tile import TileContext
    from contextlib import ExitStack

    f32 = mybir.dt.float32
    f16 = mybir.dt.float16
    Alu = mybir.AluOpType
    Act = mybir.ActivationFunctionType
    X = mybir.AxisListType.X

    nc = bass.Bass()

    zq_d = [
        nc.dram_tensor(f"zq{q}", [128, 2048], f16, kind="ExternalInput")
        for q in range(4)
    ]
    posi_d = nc.dram_tensor("posI", [128, 128], f32, kind="ExternalInput")
    mskb_d = nc.dram_tensor("mskB", [128, 128], f32, kind="ExternalInput")
    loss_d = nc.dram_tensor("loss", [128, TILES_PER_CORE], f32, kind="ExternalOutput")

    with ExitStack() as ctx:
        tc = ctx.enter_context(TileContext(nc))
        const = ctx.enter_context(tc.tile_pool(name="const", bufs=1))
        egp = ctx.enter_context(tc.tile_pool(name="egp", bufs=2))
        psum = ctx.enter_context(
            tc.tile_pool(name="psum", bufs=4, space=bass.MemorySpace.PSUM)
        )
        stats = ctx.enter_context(tc.tile_pool(name="stats", bufs=3))

        zqt = []
        for q in range(4):
            zt = const.tile([128, 2048], f16, tag=f"zq{q}")
            nc.gpsimd.dma_start(out=zt, in_=zq_d[q][:])
            zqt.append(zt)
        posit = const.tile([128, 128], f32, tag="posI")
        nc.gpsimd.dma_start(out=posit, in_=posi_d[:])
        mskbt = const.tile([128, 128], f32, tag="mskB")
        nc.gpsimd.dma_start(out=mskbt, in_=mskb_d[:])
        losst = const.tile([128, TILES_PER_CORE], f32, tag="losst")

        def chunk_matmuls(P, t, e):
            # chunk e covers global cols [e*CHUNK, (e+1)*CHUNK)
            lhsT = zqt[0][:, t * 128 : (t + 1) * 128]
            dj = (t * 128) // 512  # 512-piece of chunk 0 containing self-diag
            for j in range(2):
                gcol = e * CHUNK + j * 512
                is_diag_chunk = e == 0 and j == dj
                nc.tensor.matmul(
                    P[:, j * 512 : (j + 1) * 512],
                    lhsT,
                    zqt[gcol // 2048][:, gcol % 2048 : gcol % 2048 + 512],
                    start=True,
                    stop=not is_diag_chunk,
                    skip_group_check=True,
                )
                if is_diag_chunk:
                    # self-diag block += -1e5*I  (I.T @ (-1e5*I) accumulated)
                    nc.tensor.matmul(
                        P[:, t * 128 : t * 128 + 128],
                        posit,
                        mskbt,
                        start=False,
                        stop=True,
                        skip_group_check=True,
                    )

        POS_E = 4096 // CHUNK  # chunk holding the positive-pair diagonal
        for t in range(TILES_PER_CORE):
            nm = stats.tile([128, NCHUNK], f32, tag="nm")   # -m_e per chunk
            ssq = stats.tile([128, NCHUNK], f32, tag="ssq")  # sum e^{x-m_e}
            pos = stats.tile([128, 1], f32, tag="pos")
            scr = stats.tile([128, 128], f32, tag="scr")

            for e in range(NCHUNK):
                P = psum.tile([128, CHUNK], f32, tag="P")
                chunk_matmuls(P, t, e)
                if e == POS_E:
                    # positive-pair diag at chunk-local cols [t*128, +128)
                    nc.vector.scalar_tensor_tensor(
                        out=scr,
                        in0=P[:, t * 128 : t * 128 + 128],
                        scalar=1.0,
                        in1=posit,
                        op0=Alu.mult,
                        op1=Alu.mult,
                        accum_out=pos,
                    )
                nc.vector.reduce_max(out=nm[:, e : e + 1], in_=P, axis=X, negate=True)
                eg = egp.tile([128, CHUNK], f32, tag="eg")
                nc.scalar.activation(
                    out=eg,
                    in_=P,
                    func=Act.Exp,
                    bias=nm[:, e : e + 1],
                    scale=1.0,
                    accum_out=ssq[:, e : e + 1],
                )

            # tail: exact recombination of the four quarter-shifted sums
            # m = global row max = -min_q nm_q  (mt := -m)
            mt = stats.tile([128, 1], f32, tag="mt")
            nc.vector.tensor_reduce(out=mt, in_=nm, axis=X, op=Alu.min)
            d = stats.tile([128, NCHUNK], f32, tag="d")
            nc.vector.tensor_scalar(
                out=d, in0=nm, scalar1=mt, scalar2=None, op0=Alu.subtract
            )
            f = stats.tile([128, NCHUNK], f32, tag="f")
            nc.scalar.activation(out=f, in_=d, func=Act.Exp, scale=-1.0)
            s03 = stats.tile([128, 1], f32, tag="s03")
            nc.vector.scalar_tensor_tensor(
                out=scr[:, 0:NCHUNK],
                in0=ssq,
                scalar=1.0,
                in1=f,
                op0=Alu.mult,
                op1=Alu.mult,
                accum_out=s03,
            )
            pe = stats.tile([128, 1], f32, tag="pe")   # e^{pos - m}
            nc.scalar.activation(out=pe, in_=pos, func=Act.Exp, bias=mt, scale=1.0)
            stot = stats.tile([128, 1], f32, tag="stot")
            nc.vector.tensor_add(stot, s03, pe)
            # loss = ln(stot) + m - pos = lg - mt - pos
            lg = stats.tile([128, 1], f32, tag="lg")
            nc.scalar.activation(out=lg, in_=stot, func=Act.Ln)
            lp = stats.tile([128, 1], f32, tag="lp")
            nc.vector.tensor_sub(lp, lg, mt)
            nc.vector.tensor_sub(losst[:, t : t + 1], lp, pos)

        nc.gpsimd.dma_start(out=loss_d[:], in_=losst)

    _split_waits(nc)
    return nc


def _get_module():
    if "nc" not in _cached:
        _cached["nc"] = _build_module()
    return _cached["nc"]


def _host_inputs(z_i, z_j):
    z = np.concatenate(
        [np.asarray(z_i, np.float32), np.asarray(z_j, np.float32)], axis=0
    )
    s = np.float32(1.0 / np.sqrt(TEMP))
    zT = np.ascontiguousarray((z * s).T).astype(np.float16)  # [128, 8192]

    posI = np.eye(128, dtype=np.float32)
    mskB = np.float32(MASK_NEG) * np.eye(128, dtype=np.float32)

    in_maps = []
    for c in range(N_CORES):
        k = c * ROWS_PER_CORE
        rot = np.concatenate([zT[:, k:], zT[:, :k]], axis=1)
        im = {
            f"zq{q}": np.ascontiguousarray(rot[:, q * 2048 : (q + 1) * 2048])
            for q in range(4)
        }
        im["posI"] = posI
        im["mskB"] = mskB
        in_maps.append(im)
    return in_maps


def run_full(z_i, z_j, trace=False, trace_kwargs=None):
    """Run on 8 cores; returns (loss_scalar, BassKernelResults)."""
    from concourse.bass_utils import run_bass_kernel_spmd

    nc = _get_module()
    in_maps = _host_inputs(z_i, z_j)
    res = run_bass_kernel_spmd(
        nc,
        in_maps,
        core_ids=list(range(N_CORES)),
        trace=trace,
        **(trace_kwargs or {}),
    )
    total = np.float64(0.0)
    for c in range(N_CORES):
        total += res.results[c]["loss"].astype(np.float64).sum()
    loss = np.array(total / NROWS, dtype=np.float32)
    return loss, res


def kernel(z_i, z_j):
    loss, _ = run_full(z_i, z_j, trace=bool(os.environ.get("KERNEL_TRACE")))
    return loss
